# revision 16
# baseline (speedup 1.0000x reference)
"""Causal multi-head attention (B=2, S=2048, H=32, D=128) on 8 TRN2 NeuronCores.

Strategy (tensor-parallel over (batch, head) pairs — 64 pairs, 8 per core):

Host side packs per-head inputs into device-friendly layouts:
  qT, kT : [hpc, D, S]  bf16 — Q^T / K^T per head (d on partitions)
  vA     : [hpc, 128, NT*129] bf16 — V tiled [kv-tile, 129] with a ones
           column appended (col 128) so the softmax denominator falls out of
           the PV matmul as an extra output column.
  tri    : [128, 128] bf16 — tri[p, f] = 1 iff p <= f (causal keep-mask for
           diagonal 128x128 blocks in S^T layout).

Device per head:
  S^T[kv, q] tiles = K_tile^T-weights @ Q^T (PE, bf16, fp32 PSUM), packed per
  q-block (512 q columns) into PSUM banks with causal trimming; one big exp
  per PSUM wave on ACT (scale=1/sqrt(D) folded in, no max subtraction —
  scores are O(5) so exp is safe in fp32); causal diagonal fixed by a bf16
  tri-mask multiply on DVE; PV with P^T chunks as the stationary operand so
  the output lands in natural [q, d] layout and the ones column of vA
  accumulates the row sums; normalize with reciprocal + tensor_scalar on DVE.

Upper-triangle blocks are skipped entirely: exp(-1e9) underflows to exactly
0.0 in fp32, so dropping them is bit-equivalent to the reference softmax.
"""

import math

import numpy as np
import ml_dtypes

import concourse.bass as bass
import concourse.mybir as mybir
import concourse.tile as tile
from concourse import bacc
from concourse.tile_rust import add_dep_helper

B, S, H, D = 2, 2048, 32, 128
N_CORES = 8
HPC = (B * H) // N_CORES  # head-pairs per core
VW = D + 1                # V width including the ones column
SCALE = 1.0 / math.sqrt(D)
CHUNK_OFF = (0, 129, 258, 512)  # PV output chunk offsets (chunk 3 in bank 1)
BF16 = mybir.dt.bfloat16
F32 = mybir.dt.float32


def _qblock_layout(qb):
    """Bank-packed S^T layout for q-block qb (512 q cols, kv tiles 0..4qb+3).

    Returns (tiles, nbanks, valid_cols) where tiles is a list of
    (j, col, width, c0): kv-tile j lands at packed column `col`, covering
    local q columns [c0*128, c0*128 + width). Widths are causal-trimmed for
    the 4 diagonal tiles and bin-packed so no matmul crosses a PSUM bank.
    The only slack (256 cols) trails at the very end.
    """
    tiles = []
    bank = 0
    for j in range(qb * 4):
        tiles.append((j, bank * 512, 512, 0))
        bank += 1
    d0 = qb * 4
    tiles.append((d0 + 0, bank * 512, 512, 0))
    bank += 1
    tiles.append((d0 + 1, bank * 512, 384, 1))
    tiles.append((d0 + 3, bank * 512 + 384, 128, 3))
    bank += 1
    tiles.append((d0 + 2, bank * 512, 256, 2))
    bank += 1
    return tiles, bank, (bank - 1) * 512 + 256


def build_module(hpc=HPC, s=S, wave_banks=3):
    nt = s // 128
    qnb = s // 512
    ptw = ((qnb - 1) * 4 + 3) * 512  # widest packed q-block

    nc = bacc.Bacc(trn_type="TRN2")
    qT = nc.dram_tensor("qT", [hpc, D, s], BF16, kind="ExternalInput")
    kT = nc.dram_tensor("kT", [hpc, D, s], BF16, kind="ExternalInput")
    vA = nc.dram_tensor("vA", [hpc, 128, nt * VW], BF16, kind="ExternalInput")
    tri = nc.dram_tensor("tri", [128, 128], BF16, kind="ExternalInput")
    out = nc.dram_tensor("out", [hpc, 128, nt * D], F32, kind="ExternalOutput")

    exp_fn = mybir.ActivationFunctionType.Exp

    with tile.TileContext(nc) as tc:
        with (
            tc.tile_pool(name="const", bufs=1) as cpool,
            tc.tile_pool(name="io", bufs=2) as iopool,
            tc.tile_pool(name="pt", bufs=3) as ptpool,
            tc.tile_pool(name="ps", bufs=2, space="PSUM") as pspool,
            tc.tile_pool(name="po", bufs=1, space="PSUM") as popool,
            tc.tile_pool(name="nrm", bufs=4) as npool,
            tc.tile_pool(name="un", bufs=2) as unpool,
        ):
            tri_sb = cpool.tile([128, 128], BF16, tag="tri", name="tri_sb")
            nc.sync.dma_start(out=tri_sb, in_=tri[:, :])
            zw = cpool.tile([128, 128], BF16, tag="zw", name="zw")
            nc.vector.memset(zw, 0.0)

            # ---- flat wave pipeline across q-blocks and heads ----
            # Per wave: scores matmuls -> exp (ACT) -> diag tri-mask (DVE);
            # PV matmuls trail one wave behind so PE streams wave w+1's
            # scores while ACT runs exp(w) and always has PV work queued.
            # Q-blocks run in descending size so head starts have deep work.
            state = {}     # per-head SBUF tiles
            pending = []   # wave dicts awaiting PV emission (lag queue)
            PV_LAG = 3     # PV trails scores by 3 waves: its exp/tri deps are
                           # guaranteed complete (ps slot WAR), so PE never
                           # head-of-line blocks on ACT/DVE.

            def emit_scores(wv):
                st = wv["st"]
                ps = pspool.tile(
                    [128, wave_banks * 512], F32, tag="ps",
                    name=f"ps{wv['h']}_{wv['qb']}_{wv['wb']}",
                )
                for (j, col, w, c0) in wv["tiles"]:
                    lcol = col - wv["wb"] * 512
                    q0 = wv["qb"] * 512 + c0 * 128
                    nc.tensor.matmul(
                        ps[:, lcol:lcol + w],
                        st["kT"][:, j * 128:(j + 1) * 128],
                        st["qT"][:, q0:q0 + w],
                        start=True, stop=True,
                    )
                ext = min(wv["wn"] * 512, wv["valid"] - wv["wb"] * 512)
                nc.scalar.activation(
                    wv["pt"][:, wv["wb"] * 512: wv["wb"] * 512 + ext],
                    ps[:, 0:ext],
                    exp_fn, scale=SCALE,
                )
                for (j, col, w, c0) in wv["tiles"]:
                    if j >= wv["qb"] * 4:  # diagonal tile: causal mask
                        nc.vector.tensor_mul(
                            wv["pt"][:, col:col + 128],
                            wv["pt"][:, col:col + 128],
                            tri_sb,
                        )

            def emit_pv(wv):
                st = wv["st"]
                if wv["first"]:
                    # PV accumulator for this q-block. The j=0 matmuls of
                    # chunks 0 (bank 0) and 3 (bank 1) carry start=True: the
                    # bank-wide has_written clear makes every other first
                    # write to the bank overwrite-where-clear and later ones
                    # accumulate. Explicit deps pin the start matmul of bank
                    # 0 before its sibling chunks' first writes so Tile
                    # cannot reorder them ahead of the clear.
                    wv["qpo"][0] = popool.tile([128, 1024], F32, tag="po",
                                               name=f"po{wv['h']}_{wv['qb']}")
                po = wv["qpo"][0]
                for (j, col, w, c0) in wv["tiles"]:
                    for c in range(c0, 4):
                        lhsT = wv["pt"][:, col + (c - c0) * 128: col + (c - c0) * 128 + 128]
                        first_write = wv["first"] and j == 0
                        mm = nc.tensor.matmul(
                            po[:, CHUNK_OFF[c]:CHUNK_OFF[c] + VW],
                            lhsT,
                            st["vA"][:, j * VW:(j + 1) * VW],
                            start=first_write and c in (0, 3),
                            stop=False, skip_group_check=True,
                        )
                        if first_write and c == 0:
                            wv["qpo"].append(mm.ins)
                        elif first_write and c in (1, 2):
                            add_dep_helper(mm.ins, wv["qpo"][1], sync=False,
                                           reason="bank0 clear before sibling writes")
                if wv["last"]:
                    # Drain po with one copy (frees both banks), then
                    # normalize from SBUF off the critical path.
                    h, qb = wv["h"], wv["qb"]
                    un = unpool.tile([128, 1024], F32, tag="un", name=f"un{h}_{qb}")
                    nc.vector.tensor_copy(un[:, 0:CHUNK_OFF[2] + VW],
                                          po[:, 0:CHUNK_OFF[2] + VW])
                    nc.vector.tensor_copy(un[:, 512:512 + VW],
                                          po[:, 512:512 + VW])
                    for c in range(4):
                        qi = qb * 4 + c
                        rc = npool.tile([128, 1], F32, tag="rc", name=f"rc{h}_{qi}")
                        nc.vector.reciprocal(
                            rc, un[:, CHUNK_OFF[c] + D: CHUNK_OFF[c] + D + 1]
                        )
                        nc.vector.tensor_scalar_mul(
                            st["out"][:, qi * D:(qi + 1) * D],
                            un[:, CHUNK_OFF[c]:CHUNK_OFF[c] + D],
                            rc,
                        )
                    if wv["head_last"]:
                        nc.sync.dma_start(out=out[h], in_=st["out"])

            for h in range(hpc):
                # Tiny first-wave slices first (kT[:, :wave_banks*128] and the
                # last q-block's qT columns) so head 0's first matmuls start
                # as soon as possible; then the bulk.
                w0k = wave_banks * 128
                q0c = (qnb - 1) * 512
                kT_sb = iopool.tile([128, s], BF16, tag="kT", name=f"kT{h}")
                nc.sync.dma_start(out=kT_sb[:, 0:w0k], in_=kT[h][:, 0:w0k])
                qT_sb = iopool.tile([128, s], BF16, tag="qT", name=f"qT{h}")
                nc.sync.dma_start(out=qT_sb[:, q0c:s], in_=qT[h][:, q0c:s])
                nc.sync.dma_start(out=kT_sb[:, w0k:s], in_=kT[h][:, w0k:s])
                nc.sync.dma_start(out=qT_sb[:, 0:q0c], in_=qT[h][:, 0:q0c])
                vA_sb = iopool.tile([128, nt * VW], BF16, tag="vA", name=f"vA{h}")
                nc.sync.dma_start(out=vA_sb, in_=vA[h])
                out_sb = iopool.tile([128, nt * D], F32, tag="osb", name=f"osb{h}")
                st = {"kT": kT_sb, "qT": qT_sb, "vA": vA_sb, "out": out_sb}

                for qbi, qb in enumerate(range(qnb - 1, -1, -1)):
                    tiles, nbanks, valid = _qblock_layout(qb)
                    pt_sb = ptpool.tile([128, ptw], BF16, tag="pt", name=f"pt{h}_{qb}")
                    qpo = [None]  # po allocated lazily at first PV of q-block
                    wb = 0
                    qwaves = []
                    while wb < nbanks:
                        wn = min(wave_banks, nbanks - wb)
                        qwaves.append({
                            "h": h, "qb": qb, "wb": wb, "wn": wn,
                            "tiles": [t for t in tiles
                                      if wb * 512 <= t[1] < (wb + wn) * 512],
                            "valid": valid, "pt": pt_sb, "st": st, "qpo": qpo,
                            "first": wb == 0, "last": False,
                            "head_last": False,
                        })
                        wb += wn
                    qwaves[-1]["last"] = True
                    qwaves[-1]["head_last"] = qb == 0
                    for wv in qwaves:
                        emit_scores(wv)
                        pending.append(wv)
                        if len(pending) > PV_LAG:
                            emit_pv(pending.pop(0))
            for wv in pending:
                emit_pv(wv)
    nc.compile()
    return nc


def _pack_inputs(xq, xk, xv, s=S, b=B, h=H):
    """Full [B,S,H,D] fp32 inputs -> per-pair device layouts (bf16)."""
    bf16 = ml_dtypes.bfloat16
    nt = s // 128
    nh = b * h
    # [B,S,H,D] -> [B,H,S,D] -> [nh, S, D]
    q = np.transpose(np.asarray(xq), (0, 2, 1, 3)).reshape(nh, s, D)
    k = np.transpose(np.asarray(xk), (0, 2, 1, 3)).reshape(nh, s, D)
    v = np.transpose(np.asarray(xv), (0, 2, 1, 3)).reshape(nh, s, D)
    qT = np.ascontiguousarray(q.transpose(0, 2, 1)).astype(bf16)  # [nh, D, S]
    kT = np.ascontiguousarray(k.transpose(0, 2, 1)).astype(bf16)
    v4 = v.reshape(nh, nt, 128, D)
    ones = np.ones((nh, nt, 128, 1), np.float32)
    vA = np.concatenate([v4, ones], axis=3)          # [nh, nt, 128, VW]
    vA = np.ascontiguousarray(vA.transpose(0, 2, 1, 3)).reshape(nh, 128, nt * VW)
    vA = vA.astype(bf16)
    tri = np.triu(np.ones((128, 128), np.float32)).astype(bf16)
    return qT, kT, vA, tri


def _unpack_output(outs, s=S, b=B, h=H):
    """Per-core [hpc, 128, NT*D] fp32 -> [B, S, H*D]."""
    nt = s // 128
    o = np.concatenate([np.asarray(x) for x in outs], axis=0)  # [nh, 128, nt*D]
    o = o.reshape(b * h, 128, nt, D).transpose(0, 2, 1, 3)     # [nh, nt, 128, D]
    o = o.reshape(b, h, s, D).transpose(0, 2, 1, 3)            # [B, S, H, D]
    return np.ascontiguousarray(o.reshape(b, s, h * D)).astype(np.float32)


_CACHE = {}


def _get_module():
    if "nc" not in _CACHE:
        _CACHE["nc"] = build_module()
    return _CACHE["nc"]


def make_in_maps(xq, xk, xv):
    qT, kT, vA, tri = _pack_inputs(xq, xk, xv)
    in_maps = []
    for core in range(N_CORES):
        sl = slice(core * HPC, (core + 1) * HPC)
        in_maps.append({
            "qT": np.ascontiguousarray(qT[sl]),
            "kT": np.ascontiguousarray(kT[sl]),
            "vA": np.ascontiguousarray(vA[sl]),
            "tri": tri,
        })
    return in_maps


def kernel(xq, xk, xv, cache_k, cache_v, mask, start_pos):
    assert int(start_pos) == 0, "kernel specialized for start_pos == 0"
    from concourse.bass_utils import run_bass_kernel_spmd

    nc = _get_module()
    in_maps = make_in_maps(xq, xk, xv)
    res = run_bass_kernel_spmd(nc, in_maps, core_ids=list(range(N_CORES)))
    outs = [res.results[i]["out"] for i in range(N_CORES)]
    return _unpack_output(outs)


# revision 18
# speedup vs baseline: 1.0037x; 1.0037x over previous
"""Causal multi-head attention (B=2, S=2048, H=32, D=128) on 8 TRN2 NeuronCores.

Strategy (tensor-parallel over (batch, head) pairs — 64 pairs, 8 per core):

Host side packs per-head inputs into device-friendly layouts:
  qT, kT : [hpc, D, S]  bf16 — Q^T / K^T per head (d on partitions)
  vA     : [hpc, 128, NT*129] bf16 — V tiled [kv-tile, 129] with a ones
           column appended (col 128) so the softmax denominator falls out of
           the PV matmul as an extra output column.
  tri    : [128, 128] bf16 — tri[p, f] = 1 iff p <= f (causal keep-mask for
           diagonal 128x128 blocks in S^T layout).

Device per head:
  S^T[kv, q] tiles = K_tile^T-weights @ Q^T (PE, bf16, fp32 PSUM), packed per
  q-block (512 q columns) into PSUM banks with causal trimming; one big exp
  per PSUM wave on ACT (scale=1/sqrt(D) folded in, no max subtraction —
  scores are O(5) so exp is safe in fp32); causal diagonal fixed by a bf16
  tri-mask multiply on DVE; PV with P^T chunks as the stationary operand so
  the output lands in natural [q, d] layout and the ones column of vA
  accumulates the row sums; normalize with reciprocal + tensor_scalar on DVE.

Upper-triangle blocks are skipped entirely: exp(-1e9) underflows to exactly
0.0 in fp32, so dropping them is bit-equivalent to the reference softmax.
"""

import math

import numpy as np
import ml_dtypes

import concourse.bass as bass
import concourse.mybir as mybir
import concourse.tile as tile
from concourse import bacc
from concourse.tile_rust import add_dep_helper

B, S, H, D = 2, 2048, 32, 128
N_CORES = 8
HPC = (B * H) // N_CORES  # head-pairs per core
VW = D + 1                # V width including the ones column
SCALE = 1.0 / math.sqrt(D)
CHUNK_OFF = (0, 129, 258, 512)  # PV output chunk offsets (chunk 3 in bank 1)
BF16 = mybir.dt.bfloat16
F32 = mybir.dt.float32


def _qblock_layout(qb):
    """Bank-packed S^T layout for q-block qb (512 q cols, kv tiles 0..4qb+3).

    Returns (tiles, nbanks, valid_cols) where tiles is a list of
    (j, col, width, c0): kv-tile j lands at packed column `col`, covering
    local q columns [c0*128, c0*128 + width). Widths are causal-trimmed for
    the 4 diagonal tiles and bin-packed so no matmul crosses a PSUM bank.
    The only slack (256 cols) trails at the very end.
    """
    tiles = []
    bank = 0
    for j in range(qb * 4):
        tiles.append((j, bank * 512, 512, 0))
        bank += 1
    d0 = qb * 4
    tiles.append((d0 + 0, bank * 512, 512, 0))
    bank += 1
    tiles.append((d0 + 1, bank * 512, 384, 1))
    tiles.append((d0 + 3, bank * 512 + 384, 128, 3))
    bank += 1
    tiles.append((d0 + 2, bank * 512, 256, 2))
    bank += 1
    return tiles, bank, (bank - 1) * 512 + 256


def build_module(hpc=HPC, s=S, wave_banks=3):
    nt = s // 128
    qnb = s // 512
    ptw = ((qnb - 1) * 4 + 3) * 512  # widest packed q-block

    nc = bacc.Bacc(trn_type="TRN2")
    qT = nc.dram_tensor("qT", [hpc, D, s], BF16, kind="ExternalInput")
    kT = nc.dram_tensor("kT", [hpc, D, s], BF16, kind="ExternalInput")
    vA = nc.dram_tensor("vA", [hpc, 128, nt * VW], BF16, kind="ExternalInput")
    tri = nc.dram_tensor("tri", [128, 128], BF16, kind="ExternalInput")
    out = nc.dram_tensor("out", [hpc, 128, nt * D], F32, kind="ExternalOutput")

    exp_fn = mybir.ActivationFunctionType.Exp

    with tile.TileContext(nc) as tc:
        with (
            tc.tile_pool(name="const", bufs=1) as cpool,
            tc.tile_pool(name="io", bufs=2) as iopool,
            tc.tile_pool(name="pt", bufs=3) as ptpool,
            tc.tile_pool(name="ps", bufs=2, space="PSUM") as pspool,
            tc.tile_pool(name="po", bufs=1, space="PSUM") as popool,
            tc.tile_pool(name="nrm", bufs=4) as npool,
            tc.tile_pool(name="un", bufs=2) as unpool,
        ):
            tri_sb = cpool.tile([128, 128], BF16, tag="tri", name="tri_sb")
            nc.sync.dma_start(out=tri_sb, in_=tri[:, :])
            zw = cpool.tile([128, 128], BF16, tag="zw", name="zw")
            nc.vector.memset(zw, 0.0)

            # ---- flat wave pipeline across q-blocks and heads ----
            # Per wave: scores matmuls -> exp (ACT) -> diag tri-mask (DVE);
            # PV matmuls trail one wave behind so PE streams wave w+1's
            # scores while ACT runs exp(w) and always has PV work queued.
            # Q-blocks run in descending size so head starts have deep work.
            state = {}     # per-head SBUF tiles
            pending = []   # wave dicts awaiting PV emission (lag queue)
            PV_LAG = 2     # PV trails scores by 2 waves: its exp/tri deps are
                           # guaranteed complete (ps slot WAR), so PE never
                           # head-of-line blocks on ACT/DVE.

            def emit_scores(wv):
                st = wv["st"]
                ps = pspool.tile(
                    [128, wave_banks * 512], F32, tag="ps",
                    name=f"ps{wv['h']}_{wv['qb']}_{wv['wb']}",
                )
                for (j, col, w, c0) in wv["tiles"]:
                    lcol = col - wv["wb"] * 512
                    q0 = wv["qb"] * 512 + c0 * 128
                    nc.tensor.matmul(
                        ps[:, lcol:lcol + w],
                        st["kT"][:, j * 128:(j + 1) * 128],
                        st["qT"][:, q0:q0 + w],
                        start=True, stop=True,
                    )
                ext = min(wv["wn"] * 512, wv["valid"] - wv["wb"] * 512)
                nc.scalar.activation(
                    wv["pt"][:, wv["wb"] * 512: wv["wb"] * 512 + ext],
                    ps[:, 0:ext],
                    exp_fn, scale=SCALE,
                )
                for (j, col, w, c0) in wv["tiles"]:
                    if j >= wv["qb"] * 4:  # diagonal tile: causal mask
                        nc.vector.tensor_mul(
                            wv["pt"][:, col:col + 128],
                            wv["pt"][:, col:col + 128],
                            tri_sb,
                        )

            def emit_pv(wv):
                st = wv["st"]
                if wv["first"]:
                    # PV accumulator for this q-block. The j=0 matmuls of
                    # chunks 0 (bank 0) and 3 (bank 1) carry start=True: the
                    # bank-wide has_written clear makes every other first
                    # write to the bank overwrite-where-clear and later ones
                    # accumulate. Explicit deps pin the start matmul of bank
                    # 0 before its sibling chunks' first writes so Tile
                    # cannot reorder them ahead of the clear.
                    wv["qpo"][0] = popool.tile([128, 1024], F32, tag="po",
                                               name=f"po{wv['h']}_{wv['qb']}")
                po = wv["qpo"][0]
                for (j, col, w, c0) in wv["tiles"]:
                    for c in range(c0, 4):
                        lhsT = wv["pt"][:, col + (c - c0) * 128: col + (c - c0) * 128 + 128]
                        first_write = wv["first"] and j == 0
                        mm = nc.tensor.matmul(
                            po[:, CHUNK_OFF[c]:CHUNK_OFF[c] + VW],
                            lhsT,
                            st["vA"][:, j * VW:(j + 1) * VW],
                            start=first_write and c in (0, 3),
                            stop=False, skip_group_check=True,
                        )
                        if first_write and c == 0:
                            wv["qpo"].append(mm.ins)
                        elif first_write and c in (1, 2):
                            add_dep_helper(mm.ins, wv["qpo"][1], sync=False,
                                           reason="bank0 clear before sibling writes")
                if wv["last"]:
                    # Drain po with one copy (frees both banks), then
                    # normalize from SBUF off the critical path.
                    h, qb = wv["h"], wv["qb"]
                    un = unpool.tile([128, 1024], F32, tag="un", name=f"un{h}_{qb}")
                    nc.vector.tensor_copy(un[:, 0:CHUNK_OFF[2] + VW],
                                          po[:, 0:CHUNK_OFF[2] + VW])
                    nc.vector.tensor_copy(un[:, 512:512 + VW],
                                          po[:, 512:512 + VW])
                    for c in range(4):
                        qi = qb * 4 + c
                        rc = npool.tile([128, 1], F32, tag="rc", name=f"rc{h}_{qi}")
                        nc.vector.reciprocal(
                            rc, un[:, CHUNK_OFF[c] + D: CHUNK_OFF[c] + D + 1]
                        )
                        nc.vector.tensor_scalar_mul(
                            st["out"][:, qi * D:(qi + 1) * D],
                            un[:, CHUNK_OFF[c]:CHUNK_OFF[c] + D],
                            rc,
                        )
                    if wv["head_last"]:
                        nc.sync.dma_start(out=out[h], in_=st["out"])

            for h in range(hpc):
                # Tiny first-wave slices first (kT[:, :wave_banks*128] and the
                # last q-block's qT columns) so head 0's first matmuls start
                # as soon as possible; then the bulk.
                w0k = wave_banks * 128
                q0c = (qnb - 1) * 512
                kT_sb = iopool.tile([128, s], BF16, tag="kT", name=f"kT{h}")
                nc.sync.dma_start(out=kT_sb[:, 0:w0k], in_=kT[h][:, 0:w0k])
                qT_sb = iopool.tile([128, s], BF16, tag="qT", name=f"qT{h}")
                nc.sync.dma_start(out=qT_sb[:, q0c:s], in_=qT[h][:, q0c:s])
                nc.sync.dma_start(out=kT_sb[:, w0k:s], in_=kT[h][:, w0k:s])
                nc.sync.dma_start(out=qT_sb[:, 0:q0c], in_=qT[h][:, 0:q0c])
                vA_sb = iopool.tile([128, nt * VW], BF16, tag="vA", name=f"vA{h}")
                nc.sync.dma_start(out=vA_sb, in_=vA[h])
                out_sb = iopool.tile([128, nt * D], F32, tag="osb", name=f"osb{h}")
                st = {"kT": kT_sb, "qT": qT_sb, "vA": vA_sb, "out": out_sb}

                for qbi, qb in enumerate(range(qnb - 1, -1, -1)):
                    tiles, nbanks, valid = _qblock_layout(qb)
                    pt_sb = ptpool.tile([128, ptw], BF16, tag="pt", name=f"pt{h}_{qb}")
                    qpo = [None]  # po allocated lazily at first PV of q-block
                    wb = 0
                    qwaves = []
                    while wb < nbanks:
                        wn = min(wave_banks, nbanks - wb)
                        qwaves.append({
                            "h": h, "qb": qb, "wb": wb, "wn": wn,
                            "tiles": [t for t in tiles
                                      if wb * 512 <= t[1] < (wb + wn) * 512],
                            "valid": valid, "pt": pt_sb, "st": st, "qpo": qpo,
                            "first": wb == 0, "last": False,
                            "head_last": False,
                        })
                        wb += wn
                    qwaves[-1]["last"] = True
                    qwaves[-1]["head_last"] = qb == 0
                    for wv in qwaves:
                        emit_scores(wv)
                        pending.append(wv)
                        if len(pending) > PV_LAG:
                            emit_pv(pending.pop(0))
            for wv in pending:
                emit_pv(wv)
    nc.compile()
    return nc


def _pack_inputs(xq, xk, xv, s=S, b=B, h=H):
    """Full [B,S,H,D] fp32 inputs -> per-pair device layouts (bf16)."""
    bf16 = ml_dtypes.bfloat16
    nt = s // 128
    nh = b * h
    # [B,S,H,D] -> [B,H,S,D] -> [nh, S, D]
    q = np.transpose(np.asarray(xq), (0, 2, 1, 3)).reshape(nh, s, D)
    k = np.transpose(np.asarray(xk), (0, 2, 1, 3)).reshape(nh, s, D)
    v = np.transpose(np.asarray(xv), (0, 2, 1, 3)).reshape(nh, s, D)
    qT = np.ascontiguousarray(q.transpose(0, 2, 1)).astype(bf16)  # [nh, D, S]
    kT = np.ascontiguousarray(k.transpose(0, 2, 1)).astype(bf16)
    v4 = v.reshape(nh, nt, 128, D)
    ones = np.ones((nh, nt, 128, 1), np.float32)
    vA = np.concatenate([v4, ones], axis=3)          # [nh, nt, 128, VW]
    vA = np.ascontiguousarray(vA.transpose(0, 2, 1, 3)).reshape(nh, 128, nt * VW)
    vA = vA.astype(bf16)
    tri = np.triu(np.ones((128, 128), np.float32)).astype(bf16)
    return qT, kT, vA, tri


def _unpack_output(outs, s=S, b=B, h=H):
    """Per-core [hpc, 128, NT*D] fp32 -> [B, S, H*D]."""
    nt = s // 128
    o = np.concatenate([np.asarray(x) for x in outs], axis=0)  # [nh, 128, nt*D]
    o = o.reshape(b * h, 128, nt, D).transpose(0, 2, 1, 3)     # [nh, nt, 128, D]
    o = o.reshape(b, h, s, D).transpose(0, 2, 1, 3)            # [B, S, H, D]
    return np.ascontiguousarray(o.reshape(b, s, h * D)).astype(np.float32)


_CACHE = {}


def _get_module():
    if "nc" not in _CACHE:
        _CACHE["nc"] = build_module()
    return _CACHE["nc"]


def make_in_maps(xq, xk, xv):
    qT, kT, vA, tri = _pack_inputs(xq, xk, xv)
    in_maps = []
    for core in range(N_CORES):
        sl = slice(core * HPC, (core + 1) * HPC)
        in_maps.append({
            "qT": np.ascontiguousarray(qT[sl]),
            "kT": np.ascontiguousarray(kT[sl]),
            "vA": np.ascontiguousarray(vA[sl]),
            "tri": tri,
        })
    return in_maps


def kernel(xq, xk, xv, cache_k, cache_v, mask, start_pos):
    assert int(start_pos) == 0, "kernel specialized for start_pos == 0"
    from concourse.bass_utils import run_bass_kernel_spmd

    nc = _get_module()
    in_maps = make_in_maps(xq, xk, xv)
    res = None
    for attempt in range(3):
        try:
            res = run_bass_kernel_spmd(nc, in_maps, core_ids=list(range(N_CORES)))
            break
        except Exception:
            if attempt == 2:
                raise
    outs = [res.results[i]["out"] for i in range(N_CORES)]
    return _unpack_output(outs)
